# revision 20
# baseline (speedup 1.0000x reference)
"""Trainium2 Bass kernel for nn_Attention_82403242541756 (v2).

Reference semantics (with the dim-0 chunk bug):
  qkv = inputs @ W_qkv + b_qkv                  # [3, 2048, 3072]
  q, k, v = split(qkv, 3, axis=0)               # batch split! q=batch0, k=batch1, v=batch2
  each chunk [1, 2048, 3072] flat-reinterpreted to (3, 16, 2048, 64) = 48 "heads"
  scores softmax (no max needed; |scores*scale| < ~2.3), ctx, flat-reinterpret, @ W_out + b_out

Sharding (zero communication): core c takes seq rows [256c, 256c+256) of all 3
batch items -> local heads g in [6c, 6c+6), and final output rows [768c, 768c+768).

v2 design vs v1:
  - scores matmuls in fp8e4 DoubleRow (stride-0 duplicated 64-d halves -> 2x in
    the PE cost model; psum scores are 2*q.k so ACT exp uses scale/2)
  - ctx matmuls flipped to [s, d] orientation: lhsT = expT stride-16 s-columns,
    rhs = vx [t, 65] -> out free dim 65 instead of 512-chunks (2x cheaper), and
    the normalized X tile lands directly in ctx2d row layout.
  - denominators via N=1 matmuls into a dedicated psum bank.
  - out-projection: X [128 rows, 1024] -> DRAM -> 8 xbar-transpose readbacks ->
    out^T accumulation with full K=128 contraction.
  - PSUM budget: scores 4 banks (4 x [128,512] slots) + ctx 2 + den 1 +
    shared qkv-evac/outproj 1 = 8.
  - software-pipelined emission: QKV m1 and out-projections ride the PE slack
    under the ACT-bound attention windows.
"""

import sys

sys.path.insert(0, "/opt/trn_rl_repo")

import numpy as np
import ml_dtypes

from concourse import bacc, bass, mybir, tile
from concourse.bass_utils import run_bass_kernel_spmd

BF16 = mybir.dt.bfloat16
F32 = mybir.dt.float32
FP8 = mybir.dt.float8e4
AF = mybir.ActivationFunctionType
ALU = mybir.AluOpType
PM = mybir.MatmulPerfMode

P = 128
N_CORES = 8
SEQ = 2048
H = 1024
HEADS_PER_CORE = 6
ROWS = 256  # seq rows per core
SCALE = float(H) ** -0.5  # 1/32
ACT_SCALE = SCALE

_NC_CACHE = {}


def _build():
    nc = bacc.Bacc()

    xt_e = nc.declare_dram_parameter("xt", [P, 8, 768], BF16, isOutput=False)
    wq_e = nc.declare_dram_parameter("wq", [P, 8, 3072], BF16, isOutput=False)
    bq_e = nc.declare_dram_parameter("bq", [P, 3072], BF16, isOutput=False)
    wo_e = nc.declare_dram_parameter("wo", [P, 8, 1024], BF16, isOutput=False)
    bo_e = nc.declare_dram_parameter("bo", [P, 8], F32, isOutput=False)
    out_e = nc.declare_dram_parameter("outt", [1024, 768], F32, isOutput=True)

    with tile.TileContext(nc) as tc:
        with (
            tc.tile_pool(name="dram", bufs=1, space="DRAM") as dp,
            tc.tile_pool(name="w1", bufs=1) as w1p,
            tc.tile_pool(name="yps", bufs=1, space="PSUM") as yps_p,
            tc.tile_pool(name="yb", bufs=4) as ybp,
            tc.tile_pool(name="qk", bufs=4) as qkp,
            tc.tile_pool(name="vx", bufs=2) as vxp,
            tc.tile_pool(name="expp", bufs=8) as expp,
            tc.tile_pool(name="xs", bufs=2) as xsp,
            tc.tile_pool(name="xt2", bufs=2) as xtp,
            tc.tile_pool(name="rs", bufs=2) as rsp,
            tc.tile_pool(name="stg", bufs=2) as stgp,
        ):
            # DRAM staging: yq/yk padded to 128 cols for the xbar transpose
            # (pad cols never written; transposed garbage lands on unused
            # partitions 64:128 and is never consumed).
            yq = dp.tile([12288, P], BF16)
            yk = dp.tile([12288, P], BF16)
            yv = dp.tile([12288, 64], BF16)
            yq_v = yq.rearrange("(r j) d -> r j d", j=48)
            yk_v = yk.rearrange("(r j) d -> r j d", j=48)
            yv_v = yv.rearrange("(r j) d -> r j d", j=48)
            xd = [dp.tile([P, 1024], BF16, name=f"xd{l}") for l in range(6)]

            # persistent weights / biases
            xt_sb = w1p.tile([P, 8, 768], BF16)
            wq_sb = w1p.tile([P, 8, 3072], BF16)
            bq_sb = w1p.tile([P, 3072], BF16)
            wo_sb = w1p.tile([P, 8, 1024], BF16)
            bo_sb = w1p.tile([P, 8], F32)

            # ACT exp-table warmup: absorb the table load before real work
            warm = w1p.tile([P, 16], F32)
            nc.vector.memset(warm[:], 0.0)
            warm2 = w1p.tile([P, 16], F32)
            nc.scalar.activation(warm2[:], warm[:], AF.Exp, scale=1.0)

            nc.scalar.dma_start(xt_sb[:, :, 0:512], xt_e[:, :, 0:512])
            for nb in range(6):
                eng = nc.sync if nb % 2 == 0 else nc.scalar
                eng.dma_start(
                    wq_sb[:, :, 512 * nb : 512 * (nb + 1)],
                    wq_e[:, :, 512 * nb : 512 * (nb + 1)],
                )
            nc.scalar.dma_start(bq_sb[:], bq_e[:])

            # lead-phase psum pool (closed before attention pools open)
            import contextlib

            lead_es = contextlib.ExitStack()
            ldp = lead_es.enter_context(
                tc.tile_pool(name="ldps", bufs=6, space="PSUM")
            )

            # ---------------- QKV chains ----------------
            # chain (b, m, nb): rows [b*256+128m, +128), qkv cols [512nb, +512)
            def emit_qkv_mms(b, m, nb, ps_slice, part=None):
                # part: None = all, 0 = first half, 1 = second half
                lhs = xt_sb[:, :, b * 256 + 128 * m : b * 256 + 128 * (m + 1)]
                rng = {None: range(8), 0: range(4), 1: range(4, 8)}[part]
                for k in rng:
                    nc.tensor.matmul(
                        ps_slice,
                        lhsT=lhs[:, k, :],
                        rhs=wq_sb[:, k, 512 * nb : 512 * (nb + 1)],
                        start=(k == 0),
                        stop=(k == 7),
                    )

            def emit_qkv_stage(b, m, nb, ps_slice):
                if b < 2:
                    # wide staging: data cols 0:64 + zero pad 64:128 so the
                    # xbar transpose readback sees defined data. The pad is
                    # memset only on the first rotation of each pool buffer.
                    ybuf = ybp.tile([P, 8, P], BF16, tag="ybw")
                    nc.vector.memset(ybuf[:, :, 64:128], 0.0)
                    nc.vector.tensor_tensor(
                        ybuf[:, :, 0:64],
                        ps_slice.rearrange("p (j d) -> p j d", d=64),
                        bq_sb[:, 512 * nb : 512 * (nb + 1)].rearrange(
                            "p (j d) -> p j d", d=64
                        ),
                        ALU.add,
                    )
                    dst = (yq_v if b == 0 else yk_v)[
                        128 * m : 128 * (m + 1), 8 * nb : 8 * (nb + 1), :
                    ]
                    nc.sync.dma_start(dst, ybuf[:])
                else:
                    ybuf = ybp.tile([P, 512], BF16, tag="yb")
                    nc.vector.tensor_tensor(
                        ybuf[:], ps_slice, bq_sb[:, 512 * nb : 512 * (nb + 1)], ALU.add
                    )
                    nc.sync.dma_start(
                        yv_v[128 * m : 128 * (m + 1), 8 * nb : 8 * (nb + 1), :],
                        ybuf[:].rearrange("p (j d) -> p j d", d=64),
                    )

            def emit_qkv_chain(b, m, nb, ps_slice):
                emit_qkv_mms(b, m, nb, ps_slice)
                emit_qkv_stage(b, m, nb, ps_slice)

            # m0 lead-in for the q/k batches over a 6-deep psum ring
            lead_chains = [(b, 0, nb) for nb in range(6) for b in (0, 1)]
            for b, m, nb in lead_chains:
                ld = ldp.tile([P, 512], F32, tag="ld")
                emit_qkv_chain(b, m, nb, ld[:])
            lead_es.close()
            nc.sync.dma_start(xt_sb[:, :, 512:768], xt_e[:, :, 512:768])

            att_es = contextlib.ExitStack()
            scps_p = att_es.enter_context(
                tc.tile_pool(name="scps", bufs=2, space="PSUM")
            )
            ctxps_p = att_es.enter_context(
                tc.tile_pool(name="ctxps", bufs=1, space="PSUM")
            )
            denps_p = att_es.enter_context(
                tc.tile_pool(name="denps", bufs=1, space="PSUM")
            )
            ctxps = ctxps_p.tile([P, 16, 64], F32)
            denps = denps_p.tile([P, 16], F32)

            # remaining chains ride the shared yps bank, interleaved into the
            # attention windows per this static schedule (constraints: b0m1
            # complete before fe[3] (end of h1 works since fe[3] also needs
            # b1m1, done h1), b2m1 complete before vx(3) at h2-u24):
            pieces = {
                0: [(0, 1, nb) for nb in range(6)],
                1: [(1, 1, nb) for nb in range(6)],
                2: [(2, 1, nb) for nb in range(6)],
                3: [],
                4: [],
                5: [],
            }

            def emit_piece(pc):
                b, m, nb = pc
                yps = yps_p.tile([P, 512], F32, tag="yps")
                emit_qkv_chain(b, m, nb, yps[:])

            # ---------------- attention per head ----------------
            def emit_frontend(l):
                qT = qkp.tile([P, SEQ], BF16, tag="qk", name=f"qT{l}")
                nc.sync.dma_start(qT[:], yq[SEQ * l : SEQ * (l + 1), :], transpose=True)
                kT = qkp.tile([P, SEQ], BF16, tag="qk", name=f"kT{l}")
                nc.sync.dma_start(kT[:], yk[SEQ * l : SEQ * (l + 1), :], transpose=True)
                return qT, kT

            def emit_vx(l):
                vx = vxp.tile([P, 16, 65], BF16, name=f"vx{l}", tag="vx")
                nc.vector.memset(vx[:, :, 64:65], 1.0)
                nc.sync.dma_start(
                    vx[:, :, 0:64],
                    yv[SEQ * l : SEQ * (l + 1), :].rearrange(
                        "(so p) d -> p so d", p=P
                    ),
                )
                return vx

            def emit_scores_unit(l, qT, kT, tt, h, expT):
                # unit = (tt, h): scores [128 t, 1024 s], s0 = 1024h
                sct = scps_p.tile([P, 2, 512], F32, tag="sc")
                lhsT = kT[0:64, 128 * tt : 128 * (tt + 1)]
                for half in range(2):
                    s0 = 1024 * h + 512 * half
                    nc.tensor.matmul(
                        sct[:, half, :],
                        lhsT=lhsT,
                        rhs=qT[0:64, s0 : s0 + 512],
                        start=True,
                        stop=True,
                    )
                nc.scalar.activation(
                    expT[:, 1024 * h : 1024 * (h + 1)],
                    sct[:],
                    AF.Exp,
                    scale=ACT_SCALE,
                )

            def emit_ctx(l, vx, tt, expT):
                # PSUM start=True zeroes a whole 2KiB bank, so emit exactly one
                # start (first mm into the bank this head) and one stop (last
                # mm) per bank; intermediate mms rely on has_written bits.
                for j in range(16):
                    lhsT = expT.rearrange("p (i j) -> p j i", j=16)[:, j, :]
                    nc.tensor.matmul(
                        ctxps[:, j, :],
                        lhsT=lhsT,
                        rhs=vx[:, tt, 0:64],
                        start=(tt == 0 and j % 8 == 0),
                        stop=(tt == 15 and j % 8 == 7),
                    )
                    nc.tensor.matmul(
                        denps[:, j : j + 1],
                        lhsT=lhsT,
                        rhs=vx[:, tt, 64:65],
                        start=(tt == 0 and j == 0),
                        stop=(tt == 15 and j == 15),
                    )

            def emit_normalize(l):
                rec = rsp.tile([P, 16], F32, tag="rs")
                nc.vector.reciprocal(rec[:], denps[:])
                x_sb = xsp.tile([P, 16, 64], BF16, tag="xs", name=f"x{l}")
                # read each bank's j%8==0 slice last: the next head's bank-
                # clearing start=True matmul only carries a WAR against that
                # slice, so it must be the final read of the bank.
                for j in [1, 2, 3, 4, 5, 6, 7, 0]:
                    nc.vector.tensor_scalar(
                        x_sb[:, j, :],
                        ctxps[:, j, :],
                        rec[:, j : j + 1],
                        None,
                        ALU.mult,
                    )
                nc.sync.dma_start(
                    xd[l][:, 0:512],
                    x_sb[:, 0:8, :].rearrange("p j d -> p (j d)"),
                )
                for j in [9, 10, 11, 12, 13, 14, 15, 8]:
                    nc.vector.tensor_scalar(
                        x_sb[:, j, :],
                        ctxps[:, j, :],
                        rec[:, j : j + 1],
                        None,
                        ALU.mult,
                    )
                nc.sync.dma_start(
                    xd[l][:, 512:1024],
                    x_sb[:, 8:16, :].rearrange("p j d -> p (j d)"),
                )

            def emit_outproj(l, pool=None, tag="yps"):
                if pool is None:
                    pool = yps_p
                xt2 = xtp.tile([P, 8, P], BF16, tag="xt2", name=f"xt2_{l}")
                for c in range(8):
                    nc.sync.dma_start(
                        xt2[:, c, :],
                        xd[l][:, 128 * c : 128 * (c + 1)],
                        transpose=True,
                    )
                for half in range(2):
                    stg = stgp.tile([P, 4, P], F32, tag="stg", name=f"stg{l}_{half}")
                    for mi in range(4):
                        m = 4 * half + mi
                        ops = pool.tile([P, 512], F32, tag=tag, name=f"ops{l}_{m}")
                        for c in range(8):
                            nc.tensor.matmul(
                                ops[:, 0:128],
                                lhsT=wo_sb[:, c, 128 * m : 128 * (m + 1)],
                                rhs=xt2[:, c, :],
                                start=(c == 0),
                                stop=(c == 7),
                            )
                        nc.vector.tensor_scalar(
                            stg[:, mi, :], ops[:, 0:128], bo_sb[:, m : m + 1], None, ALU.add
                        )
                    nc.sync.dma_start(
                        out_e.rearrange("(hm m p) r -> p hm m r", p=P, m=4)[
                            :, half, :, 128 * l : 128 * (l + 1)
                        ],
                        stg[:],
                    )

            # ---------------- emission schedule ----------------
            # Global stream of 192 score units (6 heads x 16 tt x 2 halves).
            # ctx matmuls trail the stream via a catch-up pointer (>= 2 global
            # tts behind, bounded bursts) so the serial ACT->ctx->scores->ACT
            # chain never forms. QKV piece-chains are emitted in halves so no
            # single PE block delays the next scores unit beyond the ACT
            # double-buffer slack.
            fe = {}
            fe[0] = emit_frontend(0)
            fe[1] = emit_frontend(1)
            nc.sync.dma_start(wo_sb[:], wo_e[:])
            nc.sync.dma_start(bo_sb[:], bo_e[:])

            expTs = {}
            vx_t = {}
            pending_yps = {}

            def emit_piece_part(pc, part):
                b, m, nb = pc
                if part in (None, 0):
                    pending_yps[pc] = yps_p.tile([P, 512], F32, tag="yps", name=f"yps_{pc[0]}_{pc[1]}_{pc[2]}")
                t = pending_yps[pc]
                emit_qkv_mms(b, m, nb, t[:], part)
                if part in (None, 1):
                    emit_qkv_stage(b, m, nb, t[:])
                    del pending_yps[pc]

            # per-head unit jobs: list of callables keyed by unit index
            jobs = {l: {} for l in range(6)}

            def add_job(l, u, fn):
                jobs[l].setdefault(u, []).append(fn)

            # b2 m0: split halves over head-0 units 0..11, vx(0) at u12
            for i, nb in enumerate(range(6)):
                pc = (2, 0, nb)
                add_job(0, 2 * i, lambda pc=pc: emit_piece_part(pc, 0))
                add_job(0, 2 * i + 1, lambda pc=pc: emit_piece_part(pc, 1))
            add_job(0, 12, lambda: vx_t.__setitem__(0, emit_vx(0)))
            # b0 m1 halves in head 1 (u1..12), b2 m1 halves (u14..25)
            for i, nb in enumerate(range(6)):
                pc = (0, 1, nb)
                add_job(1, 1 + 2 * i, lambda pc=pc: emit_piece_part(pc, 0))
                add_job(1, 2 + 2 * i, lambda pc=pc: emit_piece_part(pc, 1))
            for i, nb in enumerate(range(6)):
                pc = (2, 1, nb)
                add_job(1, 14 + 2 * i, lambda pc=pc: emit_piece_part(pc, 0))
                add_job(1, 15 + 2 * i, lambda pc=pc: emit_piece_part(pc, 1))
            # b1 m1 halves in head 2 (u0..11); fe[3] right after at u13
            for i, nb in enumerate(range(6)):
                pc = (1, 1, nb)
                add_job(2, 2 * i, lambda pc=pc: emit_piece_part(pc, 0))
                add_job(2, 2 * i + 1, lambda pc=pc: emit_piece_part(pc, 1))
            add_job(2, 13, lambda: fe.__setitem__(3, emit_frontend(3)))

            def emit_ctx_g(gtt):
                l2, tt2 = divmod(gtt, 16)
                emit_ctx(l2, vx_t[l2], tt2, expTs.pop(gtt))
                if tt2 == 15:
                    emit_normalize(l2)
                    if l2 < 5:
                        emit_outproj(l2)

            nc_ptr = 0
            for l in range(6):
                qT_l, kT_l = fe[l]
                for u in range(32):
                    tt, h = divmod(u, 2)
                    gtt = 16 * l + tt
                    if h == 0:
                        expTs[gtt] = expp.tile(
                            [P, SEQ], BF16, tag="expT", name=f"expT{l}_{tt}"
                        )
                    emit_scores_unit(l, qT_l, kT_l, tt, h, expTs[gtt])
                    for fn in jobs[l].get(u, []):
                        fn()
                    # ctx catch-up: at most 2 per unit, lag >= 2 global tts
                    budget = 2
                    while (
                        budget > 0
                        and nc_ptr <= gtt - 2
                        and (nc_ptr // 16) in vx_t
                    ):
                        emit_ctx_g(nc_ptr)
                        nc_ptr += 1
                        budget -= 1
                    if u == 16 and l + 2 <= 5 and l != 1:
                        fe[l + 2] = emit_frontend(l + 2)
                    if u == 24 and l + 1 <= 5:
                        vx_t[l + 1] = emit_vx(l + 1)
            while nc_ptr <= 95:
                emit_ctx_g(nc_ptr)
                nc_ptr += 1
            att_es.close()
            with tc.tile_pool(name="tailps", bufs=4, space="PSUM") as tp:
                emit_outproj(5, pool=tp, tag="tl")

    nc.finalize()
    return nc


def _get_nc():
    if "nc" not in _NC_CACHE:
        _NC_CACHE["nc"] = _build()
    return _NC_CACHE["nc"]


def kernel(inputs, W_qkv, b_qkv, W_out, b_out, _trace=False, _trace_kwargs=None):
    bf = ml_dtypes.bfloat16
    f8 = ml_dtypes.float8_e4m3fn
    x = np.asarray(inputs, dtype=np.float32)
    Wq = np.asarray(W_qkv, dtype=np.float32)
    bq = np.asarray(b_qkv, dtype=np.float32)
    Wo = np.asarray(W_out, dtype=np.float32)
    bo = np.asarray(b_out, dtype=np.float32)

    wq_s = np.ascontiguousarray(Wq.reshape(8, P, 3072).transpose(1, 0, 2)).astype(bf)
    wo_s = np.ascontiguousarray(Wo.reshape(8, P, 1024).transpose(1, 0, 2)).astype(bf)
    bq_s = np.ascontiguousarray(np.broadcast_to(bq[None, :], (P, 3072))).astype(bf)
    bo_s = np.ascontiguousarray(bo.reshape(8, P).T).astype(np.float32)

    in_maps = []
    for c in range(N_CORES):
        xc = x[:, ROWS * c : ROWS * (c + 1), :]  # [3, 256, 1024]
        xt = (
            xc.transpose(2, 0, 1)
            .reshape(1024, 768)
            .reshape(8, P, 768)
            .transpose(1, 0, 2)
        )
        in_maps.append(
            {
                "xt": np.ascontiguousarray(xt).astype(bf),
                "wq": wq_s,
                "bq": bq_s,
                "wo": wo_s,
                "bo": bo_s,
            }
        )

    nc = _get_nc()
    kw = {}
    if _trace:
        kw["trace"] = True
        if _trace_kwargs:
            kw.update(_trace_kwargs)
    res = run_bass_kernel_spmd(nc, in_maps, core_ids=list(range(N_CORES)), **kw)
    outs = res.results

    out = np.empty((6144, 1024), dtype=np.float32)
    for c in range(N_CORES):
        out[768 * c : 768 * (c + 1), :] = np.asarray(
            outs[c]["outt"], dtype=np.float32
        ).T
    if _trace:
        kernel.last_result = res
    return out.reshape(3, SEQ, H)


# revision 21
# speedup vs baseline: 1.0274x; 1.0274x over previous
"""Trainium2 Bass kernel for nn_Attention_82403242541756 (v2).

Reference semantics (with the dim-0 chunk bug):
  qkv = inputs @ W_qkv + b_qkv                  # [3, 2048, 3072]
  q, k, v = split(qkv, 3, axis=0)               # batch split! q=batch0, k=batch1, v=batch2
  each chunk [1, 2048, 3072] flat-reinterpreted to (3, 16, 2048, 64) = 48 "heads"
  scores softmax (no max needed; |scores*scale| < ~2.3), ctx, flat-reinterpret, @ W_out + b_out

Sharding (zero communication): core c takes seq rows [256c, 256c+256) of all 3
batch items -> local heads g in [6c, 6c+6), and final output rows [768c, 768c+768).

v2 design vs v1:
  - scores matmuls in fp8e4 DoubleRow (stride-0 duplicated 64-d halves -> 2x in
    the PE cost model; psum scores are 2*q.k so ACT exp uses scale/2)
  - ctx matmuls flipped to [s, d] orientation: lhsT = expT stride-16 s-columns,
    rhs = vx [t, 65] -> out free dim 65 instead of 512-chunks (2x cheaper), and
    the normalized X tile lands directly in ctx2d row layout.
  - denominators via N=1 matmuls into a dedicated psum bank.
  - out-projection: X [128 rows, 1024] -> DRAM -> 8 xbar-transpose readbacks ->
    out^T accumulation with full K=128 contraction.
  - PSUM budget: scores 4 banks (4 x [128,512] slots) + ctx 2 + den 1 +
    shared qkv-evac/outproj 1 = 8.
  - software-pipelined emission: QKV m1 and out-projections ride the PE slack
    under the ACT-bound attention windows.
"""

import sys

sys.path.insert(0, "/opt/trn_rl_repo")

import numpy as np
import ml_dtypes

from concourse import bacc, bass, mybir, tile
from concourse.bass_utils import run_bass_kernel_spmd

BF16 = mybir.dt.bfloat16
F32 = mybir.dt.float32
FP8 = mybir.dt.float8e4
AF = mybir.ActivationFunctionType
ALU = mybir.AluOpType
PM = mybir.MatmulPerfMode

P = 128
N_CORES = 8
SEQ = 2048
H = 1024
HEADS_PER_CORE = 6
ROWS = 256  # seq rows per core
SCALE = float(H) ** -0.5  # 1/32
ACT_SCALE = SCALE

_NC_CACHE = {}


def _build():
    nc = bacc.Bacc()

    xt_e = nc.declare_dram_parameter("xt", [P, 8, 768], BF16, isOutput=False)
    wq_e = nc.declare_dram_parameter("wq", [P, 8, 3072], BF16, isOutput=False)
    bq_e = nc.declare_dram_parameter("bq", [P, 3072], BF16, isOutput=False)
    wo_e = nc.declare_dram_parameter("wo", [P, 8, 1024], BF16, isOutput=False)
    bo_e = nc.declare_dram_parameter("bo", [P, 8], F32, isOutput=False)
    out_e = nc.declare_dram_parameter("outt", [1024, 768], F32, isOutput=True)

    with tile.TileContext(nc) as tc:
        with (
            tc.tile_pool(name="dram", bufs=1, space="DRAM") as dp,
            tc.tile_pool(name="w1", bufs=1) as w1p,
            tc.tile_pool(name="yps", bufs=1, space="PSUM") as yps_p,
            tc.tile_pool(name="yb", bufs=4) as ybp,
            tc.tile_pool(name="qk", bufs=4) as qkp,
            tc.tile_pool(name="vx", bufs=2) as vxp,
            tc.tile_pool(name="expp", bufs=8) as expp,
            tc.tile_pool(name="xs", bufs=2) as xsp,
            tc.tile_pool(name="xt2", bufs=2) as xtp,
            tc.tile_pool(name="rs", bufs=2) as rsp,
            tc.tile_pool(name="stg", bufs=2) as stgp,
        ):
            # DRAM staging: yq/yk padded to 128 cols for the xbar transpose
            # (pad cols never written; transposed garbage lands on unused
            # partitions 64:128 and is never consumed).
            yq = dp.tile([12288, P], BF16)
            yk = dp.tile([12288, P], BF16)
            yv = dp.tile([12288, 64], BF16)
            yq_v = yq.rearrange("(r j) d -> r j d", j=48)
            yk_v = yk.rearrange("(r j) d -> r j d", j=48)
            yv_v = yv.rearrange("(r j) d -> r j d", j=48)
            xd = [dp.tile([P, 1024], BF16, name=f"xd{l}") for l in range(6)]

            # persistent weights / biases
            xt_sb = w1p.tile([P, 8, 768], BF16)
            wq_sb = w1p.tile([P, 8, 3072], BF16)
            bq_sb = w1p.tile([P, 3072], BF16)
            wo_sb = w1p.tile([P, 8, 1024], BF16)
            bo_sb = w1p.tile([P, 8], F32)

            # ACT exp-table warmup: absorb the table load before real work
            warm = w1p.tile([P, 16], F32)
            nc.vector.memset(warm[:], 0.0)
            warm2 = w1p.tile([P, 16], F32)
            nc.scalar.activation(warm2[:], warm[:], AF.Exp, scale=1.0)

            nc.scalar.dma_start(xt_sb[:, :, 0:512], xt_e[:, :, 0:512])
            for nb in range(6):
                eng = nc.sync if nb % 2 == 0 else nc.scalar
                eng.dma_start(
                    wq_sb[:, :, 512 * nb : 512 * (nb + 1)],
                    wq_e[:, :, 512 * nb : 512 * (nb + 1)],
                )
            nc.scalar.dma_start(bq_sb[:], bq_e[:])

            # lead-phase psum pool (closed before attention pools open)
            import contextlib

            lead_es = contextlib.ExitStack()
            ldp = lead_es.enter_context(
                tc.tile_pool(name="ldps", bufs=6, space="PSUM")
            )

            # ---------------- QKV chains ----------------
            # chain (b, m, nb): rows [b*256+128m, +128), qkv cols [512nb, +512)
            def emit_qkv_mms(b, m, nb, ps_slice, part=None):
                # part: None = all, 0..3 = quarters (2 k-steps each)
                lhs = xt_sb[:, :, b * 256 + 128 * m : b * 256 + 128 * (m + 1)]
                rng = range(8) if part is None else range(2 * part, 2 * part + 2)
                for k in rng:
                    nc.tensor.matmul(
                        ps_slice,
                        lhsT=lhs[:, k, :],
                        rhs=wq_sb[:, k, 512 * nb : 512 * (nb + 1)],
                        start=(k == 0),
                        stop=(k == 7),
                    )

            def emit_qkv_stage(b, m, nb, ps_slice):
                if b < 2:
                    # wide staging: data cols 0:64 + zero pad 64:128 so the
                    # xbar transpose readback sees defined data. The pad is
                    # memset only on the first rotation of each pool buffer.
                    ybuf = ybp.tile([P, 8, P], BF16, tag="ybw")
                    nc.vector.memset(ybuf[:, :, 64:128], 0.0)
                    nc.vector.tensor_tensor(
                        ybuf[:, :, 0:64],
                        ps_slice.rearrange("p (j d) -> p j d", d=64),
                        bq_sb[:, 512 * nb : 512 * (nb + 1)].rearrange(
                            "p (j d) -> p j d", d=64
                        ),
                        ALU.add,
                    )
                    dst = (yq_v if b == 0 else yk_v)[
                        128 * m : 128 * (m + 1), 8 * nb : 8 * (nb + 1), :
                    ]
                    nc.sync.dma_start(dst, ybuf[:])
                else:
                    ybuf = ybp.tile([P, 512], BF16, tag="yb")
                    nc.vector.tensor_tensor(
                        ybuf[:], ps_slice, bq_sb[:, 512 * nb : 512 * (nb + 1)], ALU.add
                    )
                    nc.sync.dma_start(
                        yv_v[128 * m : 128 * (m + 1), 8 * nb : 8 * (nb + 1), :],
                        ybuf[:].rearrange("p (j d) -> p j d", d=64),
                    )

            def emit_qkv_chain(b, m, nb, ps_slice):
                emit_qkv_mms(b, m, nb, ps_slice)
                emit_qkv_stage(b, m, nb, ps_slice)

            # m0 lead-in for the q/k batches over a 6-deep psum ring
            lead_chains = [(b, 0, nb) for nb in range(6) for b in (0, 1)]
            for b, m, nb in lead_chains:
                ld = ldp.tile([P, 512], F32, tag="ld")
                emit_qkv_chain(b, m, nb, ld[:])
            lead_es.close()
            nc.sync.dma_start(xt_sb[:, :, 512:768], xt_e[:, :, 512:768])

            att_es = contextlib.ExitStack()
            scps_p = att_es.enter_context(
                tc.tile_pool(name="scps", bufs=2, space="PSUM")
            )
            ctxps_p = att_es.enter_context(
                tc.tile_pool(name="ctxps", bufs=1, space="PSUM")
            )
            denps_p = att_es.enter_context(
                tc.tile_pool(name="denps", bufs=1, space="PSUM")
            )
            ctxps = ctxps_p.tile([P, 16, 64], F32)
            denps = denps_p.tile([P, 16], F32)

            # remaining chains ride the shared yps bank, interleaved into the
            # attention windows per this static schedule (constraints: b0m1
            # complete before fe[3] (end of h1 works since fe[3] also needs
            # b1m1, done h1), b2m1 complete before vx(3) at h2-u24):
            pieces = {
                0: [(0, 1, nb) for nb in range(6)],
                1: [(1, 1, nb) for nb in range(6)],
                2: [(2, 1, nb) for nb in range(6)],
                3: [],
                4: [],
                5: [],
            }

            def emit_piece(pc):
                b, m, nb = pc
                yps = yps_p.tile([P, 512], F32, tag="yps")
                emit_qkv_chain(b, m, nb, yps[:])

            # ---------------- attention per head ----------------
            def emit_frontend(l):
                qT = qkp.tile([P, SEQ], BF16, tag="qk", name=f"qT{l}")
                nc.sync.dma_start(qT[:], yq[SEQ * l : SEQ * (l + 1), :], transpose=True)
                kT = qkp.tile([P, SEQ], BF16, tag="qk", name=f"kT{l}")
                nc.sync.dma_start(kT[:], yk[SEQ * l : SEQ * (l + 1), :], transpose=True)
                return qT, kT

            def emit_vx(l):
                vx = vxp.tile([P, 16, 65], BF16, name=f"vx{l}", tag="vx")
                nc.vector.memset(vx[:, :, 64:65], 1.0)
                nc.sync.dma_start(
                    vx[:, :, 0:64],
                    yv[SEQ * l : SEQ * (l + 1), :].rearrange(
                        "(so p) d -> p so d", p=P
                    ),
                )
                return vx

            def emit_scores_unit(l, qT, kT, tt, h, expT):
                # unit = (tt, h): scores [128 t, 1024 s], s0 = 1024h
                sct = scps_p.tile([P, 2, 512], F32, tag="sc")
                lhsT = kT[0:64, 128 * tt : 128 * (tt + 1)]
                for half in range(2):
                    s0 = 1024 * h + 512 * half
                    nc.tensor.matmul(
                        sct[:, half, :],
                        lhsT=lhsT,
                        rhs=qT[0:64, s0 : s0 + 512],
                        start=True,
                        stop=True,
                    )
                nc.scalar.activation(
                    expT[:, 1024 * h : 1024 * (h + 1)],
                    sct[:],
                    AF.Exp,
                    scale=ACT_SCALE,
                )

            def emit_ctx(l, vx, tt, expT):
                # PSUM start=True zeroes a whole 2KiB bank, so emit exactly one
                # start (first mm into the bank this head) and one stop (last
                # mm) per bank; intermediate mms rely on has_written bits.
                for j in range(16):
                    lhsT = expT.rearrange("p (i j) -> p j i", j=16)[:, j, :]
                    nc.tensor.matmul(
                        ctxps[:, j, :],
                        lhsT=lhsT,
                        rhs=vx[:, tt, 0:64],
                        start=(tt == 0 and j % 8 == 0),
                        stop=(tt == 15 and j % 8 == 7),
                    )
                    nc.tensor.matmul(
                        denps[:, j : j + 1],
                        lhsT=lhsT,
                        rhs=vx[:, tt, 64:65],
                        start=(tt == 0 and j == 0),
                        stop=(tt == 15 and j == 15),
                    )

            def emit_normalize(l):
                rec = rsp.tile([P, 16], F32, tag="rs")
                nc.vector.reciprocal(rec[:], denps[:])
                x_sb = xsp.tile([P, 16, 64], BF16, tag="xs", name=f"x{l}")
                # read each bank's j%8==0 slice last: the next head's bank-
                # clearing start=True matmul only carries a WAR against that
                # slice, so it must be the final read of the bank.
                for j in [1, 2, 3, 4, 5, 6, 7, 0]:
                    nc.vector.tensor_scalar(
                        x_sb[:, j, :],
                        ctxps[:, j, :],
                        rec[:, j : j + 1],
                        None,
                        ALU.mult,
                    )
                nc.sync.dma_start(
                    xd[l][:, 0:512],
                    x_sb[:, 0:8, :].rearrange("p j d -> p (j d)"),
                )
                for j in [9, 10, 11, 12, 13, 14, 15, 8]:
                    nc.vector.tensor_scalar(
                        x_sb[:, j, :],
                        ctxps[:, j, :],
                        rec[:, j : j + 1],
                        None,
                        ALU.mult,
                    )
                nc.sync.dma_start(
                    xd[l][:, 512:1024],
                    x_sb[:, 8:16, :].rearrange("p j d -> p (j d)"),
                )

            def emit_outproj(l, pool=None, tag="yps"):
                if pool is None:
                    pool = yps_p
                xt2 = xtp.tile([P, 8, P], BF16, tag="xt2", name=f"xt2_{l}")
                for c in range(8):
                    nc.sync.dma_start(
                        xt2[:, c, :],
                        xd[l][:, 128 * c : 128 * (c + 1)],
                        transpose=True,
                    )
                for half in range(2):
                    stg = stgp.tile([P, 4, P], F32, tag="stg", name=f"stg{l}_{half}")
                    for mi in range(4):
                        m = 4 * half + mi
                        ops = pool.tile([P, 512], F32, tag=tag, name=f"ops{l}_{m}")
                        for c in range(8):
                            nc.tensor.matmul(
                                ops[:, 0:128],
                                lhsT=wo_sb[:, c, 128 * m : 128 * (m + 1)],
                                rhs=xt2[:, c, :],
                                start=(c == 0),
                                stop=(c == 7),
                            )
                        nc.vector.tensor_scalar(
                            stg[:, mi, :], ops[:, 0:128], bo_sb[:, m : m + 1], None, ALU.add
                        )
                    nc.sync.dma_start(
                        out_e.rearrange("(hm m p) r -> p hm m r", p=P, m=4)[
                            :, half, :, 128 * l : 128 * (l + 1)
                        ],
                        stg[:],
                    )

            # ---------------- emission schedule ----------------
            # Global stream of 192 score units (6 heads x 16 tt x 2 halves).
            # ctx matmuls trail the stream via a catch-up pointer (>= 2 global
            # tts behind, bounded bursts) so the serial ACT->ctx->scores->ACT
            # chain never forms. QKV piece-chains are emitted in halves so no
            # single PE block delays the next scores unit beyond the ACT
            # double-buffer slack.
            fe = {}
            fe[0] = emit_frontend(0)
            fe[1] = emit_frontend(1)
            nc.sync.dma_start(wo_sb[:], wo_e[:])
            nc.sync.dma_start(bo_sb[:], bo_e[:])

            expTs = {}
            vx_t = {}
            pending_yps = {}

            def emit_piece_part(pc, part):
                b, m, nb = pc
                if part in (None, 0):
                    pending_yps[pc] = yps_p.tile(
                        [P, 512], F32, tag="yps",
                        name=f"yps_{pc[0]}_{pc[1]}_{pc[2]}",
                    )
                t = pending_yps[pc]
                emit_qkv_mms(b, m, nb, t[:], part)
                if part in (None, 3):
                    emit_qkv_stage(b, m, nb, t[:])
                    del pending_yps[pc]

            # per-head unit jobs: list of callables keyed by unit index
            jobs = {l: {} for l in range(6)}

            def add_job(l, u, fn):
                jobs[l].setdefault(u, []).append(fn)

            # Piece placement (quarter chains, 2 matmuls each):
            #   h0 u0..11: b2m0 x24 quarters, 2/unit (no ctx in that window)
            #   h0 u13..31 + h1 u1..5: b0m1 x24 quarters, 1/unit
            #   h1 u6..29: b1m1 x24 quarters  -> fe[3] at h2 u13
            #   h2 u0..23: b2m1 x24 quarters  -> vx(3) at h2 u24
            def quarters(bm):
                b, m = bm
                return [((b, m, nb), q) for nb in range(6) for q in range(4)]

            for i, (pc, q) in enumerate(quarters((2, 0))):
                add_job(0, i // 2, lambda pc=pc, q=q: emit_piece_part(pc, q))
            add_job(0, 12, lambda: vx_t.__setitem__(0, emit_vx(0)))
            for i, (pc, q) in enumerate(quarters((0, 1))):
                if i < 19:
                    add_job(0, 13 + i, lambda pc=pc, q=q: emit_piece_part(pc, q))
                else:
                    add_job(1, 1 + (i - 19), lambda pc=pc, q=q: emit_piece_part(pc, q))
            for i, (pc, q) in enumerate(quarters((1, 1))):
                add_job(1, 6 + i, lambda pc=pc, q=q: emit_piece_part(pc, q))
            for i, (pc, q) in enumerate(quarters((2, 1))):
                add_job(2, i, lambda pc=pc, q=q: emit_piece_part(pc, q))
            add_job(2, 13, lambda: fe.__setitem__(3, emit_frontend(3)))

            def emit_ctx_g(gtt):
                l2, tt2 = divmod(gtt, 16)
                emit_ctx(l2, vx_t[l2], tt2, expTs.pop(gtt))
                if tt2 == 15:
                    emit_normalize(l2)
                    if l2 < 5:
                        emit_outproj(l2)

            nc_ptr = 0
            for l in range(6):
                qT_l, kT_l = fe[l]
                for u in range(32):
                    tt, h = divmod(u, 2)
                    gtt = 16 * l + tt
                    if h == 0:
                        expTs[gtt] = expp.tile(
                            [P, SEQ], BF16, tag="expT", name=f"expT{l}_{tt}"
                        )
                    emit_scores_unit(l, qT_l, kT_l, tt, h, expTs[gtt])
                    for fn in jobs[l].get(u, []):
                        fn()
                    # ctx catch-up: at most 2 per unit, lag >= 2 global tts
                    budget = 2
                    while (
                        budget > 0
                        and nc_ptr <= gtt - 2
                        and (nc_ptr // 16) in vx_t
                    ):
                        emit_ctx_g(nc_ptr)
                        nc_ptr += 1
                        budget -= 1
                    if u == 16 and l + 2 <= 5 and l != 1:
                        fe[l + 2] = emit_frontend(l + 2)
                    if u == 24 and l + 1 <= 5:
                        vx_t[l + 1] = emit_vx(l + 1)
            while nc_ptr <= 95:
                emit_ctx_g(nc_ptr)
                nc_ptr += 1
            att_es.close()
            with tc.tile_pool(name="tailps", bufs=4, space="PSUM") as tp:
                emit_outproj(5, pool=tp, tag="tl")

    nc.finalize()
    return nc


def _get_nc():
    if "nc" not in _NC_CACHE:
        _NC_CACHE["nc"] = _build()
    return _NC_CACHE["nc"]


def kernel(inputs, W_qkv, b_qkv, W_out, b_out, _trace=False, _trace_kwargs=None):
    bf = ml_dtypes.bfloat16
    f8 = ml_dtypes.float8_e4m3fn
    x = np.asarray(inputs, dtype=np.float32)
    Wq = np.asarray(W_qkv, dtype=np.float32)
    bq = np.asarray(b_qkv, dtype=np.float32)
    Wo = np.asarray(W_out, dtype=np.float32)
    bo = np.asarray(b_out, dtype=np.float32)

    wq_s = np.ascontiguousarray(Wq.reshape(8, P, 3072).transpose(1, 0, 2)).astype(bf)
    wo_s = np.ascontiguousarray(Wo.reshape(8, P, 1024).transpose(1, 0, 2)).astype(bf)
    bq_s = np.ascontiguousarray(np.broadcast_to(bq[None, :], (P, 3072))).astype(bf)
    bo_s = np.ascontiguousarray(bo.reshape(8, P).T).astype(np.float32)

    in_maps = []
    for c in range(N_CORES):
        xc = x[:, ROWS * c : ROWS * (c + 1), :]  # [3, 256, 1024]
        xt = (
            xc.transpose(2, 0, 1)
            .reshape(1024, 768)
            .reshape(8, P, 768)
            .transpose(1, 0, 2)
        )
        in_maps.append(
            {
                "xt": np.ascontiguousarray(xt).astype(bf),
                "wq": wq_s,
                "bq": bq_s,
                "wo": wo_s,
                "bo": bo_s,
            }
        )

    nc = _get_nc()
    kw = {}
    if _trace:
        kw["trace"] = True
        if _trace_kwargs:
            kw.update(_trace_kwargs)
    res = run_bass_kernel_spmd(nc, in_maps, core_ids=list(range(N_CORES)), **kw)
    outs = res.results

    out = np.empty((6144, 1024), dtype=np.float32)
    for c in range(N_CORES):
        out[768 * c : 768 * (c + 1), :] = np.asarray(
            outs[c]["outt"], dtype=np.float32
        ).T
    if _trace:
        kernel.last_result = res
    return out.reshape(3, SEQ, H)
